# revision 1
# baseline (speedup 1.0000x reference)
"""Trainium2 Bass kernel for the rank-1 conv-attention block (ANRB).

Math: q,k,v = 7x7 VALID conv(x) -> [B, 8100] scalar maps; sim = q (x) k is
rank-1, so attention ctx_i = f(q_i) is a 1-D function of the scalar q_i:
    f(s) = sum_j exp(s*k_j - m(s)) * v_j / sum_j exp(s*k_j - m(s)),
    m(s) = max(s*k_max, s*k_min)   (exact row max for rank-1 logits).
We evaluate f exactly on a G=256 grid (rational warp of q), then linearly
interpolate per pixel: O(G*N) instead of O(N^2) exp evals. Epilogue: 1x1
conv (outer product), bilinear 90->96 resize (two matmuls vs precomputed
resize weights), residual add.

Sharding: core = (batch b, band g). Conv + grid table replicated per batch
across its 4 band cores (no collectives); epilogue band-parallel via
per-core input VALUES (identical SPMD program).
"""

import os
import numpy as np
import concourse.bass as bass
import concourse.mybir as mybir
import concourse.bacc as bacc
import concourse.tile as tile
from concourse.bass_utils import run_bass_kernel_spmd

FP = mybir.dt.float32
FPR = mybir.dt.float32r
BF = mybir.dt.bfloat16
CONVDT = BF if os.environ.get('KCONVDT', 'f32r') == 'bf16' else FPR
AF = mybir.ActivationFunctionType
ALU = mybir.AluOpType
AX = mybir.AxisListType

# geometry
B, C, H, W = 2, 64, 96, 96
HP = WP = 90
N = HP * WP              # 8100
XBF = 9378               # padded x free size per channel (>= 96 + XTF)
WIN1 = 8704              # stage-1 window >= 8646 (= 90*96 + 6 halo)
XTF = 9282               # x tile free (8704-1 + 576 + 1 + pad)
WINO = HP * W            # 8640 stage-2 out cols
NPAD = 8448              # padded j extent (66*128, >= 8100)
G = 256
C0 = 4.0
EPS = 0.6
ALPHA = (G - 1 - 2 * EPS) / (2 * C0)
BETA = (G - 1) / 2.0
BAND = 24
NEQ = NPAD // 8          # 1056 interp elements per Q7 core
JH = N // 4              # 2025 j per grid chunk

_NC_CACHE = {}
_LAST_RES = None
import os
PHASE = int(os.environ.get("KPHASE", "4"))


def _resize_mat(n_in, n_out):
    A = np.zeros((n_out, n_in), np.float32)
    for o in range(n_out):
        s = (o + 0.5) * n_in / n_out - 0.5
        i0 = int(np.floor(s))
        w1 = np.float32(s - i0)
        A[o, min(max(i0, 0), n_in - 1)] += 1 - w1
        A[o, min(max(i0 + 1, 0), n_in - 1)] += w1
    return A


def _grid_values():
    tt = np.arange(G, dtype=np.float64)
    t = (tt - BETA) / ALPHA
    return (t / (1 - np.abs(t) / C0)).astype(np.float32)


class _PhaseDone(Exception):
    pass


def build_nc():
    nc = bacc.Bacc(None, target_bir_lowering=False)

    xb = nc.dram_tensor("xb", [C, XBF], CONVDT, kind="ExternalInput")
    xband_d = nc.dram_tensor("xband", [C, BAND * W], FP, kind="ExternalInput")
    w2_d = nc.dram_tensor("w2", [4, 128, 21], CONVDT, kind="ExternalInput")
    gvec_d = nc.dram_tensor("gvec", [1, G], FP, kind="ExternalInput")
    gcol_d = nc.dram_tensor("gcol", [128, 2], FP, kind="ExternalInput")
    selw_d = nc.dram_tensor("selw", [21, 21], CONVDT, kind="ExternalInput")
    ahb_d = nc.dram_tensor("ahb90", [HP, BAND], FP, kind="ExternalInput")
    aw_d = nc.dram_tensor("aw", [HP, W], FP, kind="ExternalInput")
    wout_d = nc.dram_tensor("wout", [1, C], FP, kind="ExternalInput")
    bout_d = nc.dram_tensor("bout", [C, 1], FP, kind="ExternalInput")
    bq_d = nc.dram_tensor("bq128", [128, 1], FP, kind="ExternalInput")
    bv_d = nc.dram_tensor("bv128", [128, 1], FP, kind="ExternalInput")
    out_d = nc.dram_tensor("out", [C, BAND * W], FP, kind="ExternalOutput")

    with tile.TileContext(nc) as tc:
        with (
            tc.tile_pool(name="big", bufs=1) as big,
            tc.tile_pool(name="small", bufs=1) as small,
            tc.tile_pool(name="dram", bufs=1, space="DRAM") as dpool,
        ):
          try:
            # ---------------- conv phase ----------------
            xt = big.tile([128, XTF], CONVDT, tag="xt")
            xflat = xb.rearrange("c e -> (c e)")
            for t2 in range(2):
                nc.sync.dma_start(
                    xt[t2 * C:(t2 + 1) * C, :],
                    bass.AP(xflat.tensor, xflat.offset + 96 * t2,
                            [[XBF, C], [1, XTF]]))

            w2t = small.tile([128, 84], CONVDT, tag="w2t")
            for g4 in range(4):
                nc.sync.dma_start(w2t[:, 21 * g4: 21 * g4 + 21], w2_d[g4])
            selw_t = small.tile([21, 21], CONVDT, tag="selw")
            nc.sync.dma_start(selw_t[:], selw_d[:])

            s1 = big.tile([128, WIN1 + 8], CONVDT, tag="s1rep")
            qkv = big.tile([128, WINO], FP, tag="qkv")

            with tc.tile_pool(name="ps1", bufs=2, space="PSUM") as ps1pool, \
                 tc.tile_pool(name="ps2", bufs=2, space="PSUM") as ps2pool:
                CH1 = 1024
                n_ch1 = (WIN1 + CH1 - 1) // CH1
                for ch in range(n_ch1):
                    o0 = ch * CH1
                    cw = min(CH1, WIN1 - o0)
                    p1 = ps1pool.tile([128, CH1], FP, tag="p1")
                    nmm = (cw + 511) // 512
                    for g4 in range(4):
                        goff = 2 * g4 * W
                        kk = 128 if g4 < 3 else 64
                        for s in range(nmm):
                            nsz = min(512, cw - s * 512)
                            nc.tensor.matmul(
                                p1[:21, s * 512: s * 512 + nsz],
                                w2t[:kk, 21 * g4: 21 * g4 + 21],
                                xt[:kk, goff + o0 + s * 512:
                                   goff + o0 + s * 512 + nsz],
                                start=(g4 == 0), stop=(g4 == 3),
                            )
                    h = cw // 2
                    nc.scalar.copy(s1[:21, o0: o0 + h], p1[:21, :h])
                    nc.vector.tensor_copy(s1[:21, o0 + h: o0 + cw],
                                          p1[:21, h:cw])

                CH2 = 1024
                n_ch2 = (WINO + CH2 - 1) // CH2
                for ch in range(n_ch2):
                    o0 = ch * CH2
                    cw = min(CH2, WINO - o0)
                    p2 = ps2pool.tile([128, CH2], FP, tag="p2")
                    nmm = (cw + 511) // 512
                    for s in range(nmm):
                        nsz = min(512, cw - s * 512)
                        for dx in range(7):
                            nc.tensor.matmul(
                                p2[:3, s * 512: s * 512 + nsz],
                                selw_t[:21, 3 * dx: 3 * dx + 3],
                                s1[:21, o0 + s * 512 + dx:
                                   o0 + s * 512 + dx + nsz],
                                start=(dx == 0), stop=(dx == 6),
                            )
                    h = cw // 2
                    nc.scalar.copy(qkv[:3, o0: o0 + h], p2[:3, :h])
                    nc.vector.tensor_copy(qkv[:3, o0 + h: o0 + cw],
                                          p2[:3, h:cw])

            # compact bounce -> dram [3, NPAD] (pad zeros beyond 8100)
            qkv_dr = dpool.tile([3, NPAD], FP, tag="qkv_dr")
            zpad = small.tile([3, NPAD - N], FP, tag="zpad")
            nc.vector.memset(zpad[:], 0.0)
            nc.sync.dma_start(qkv_dr[:, N:], zpad[:])
            nc.sync.dma_start(
                qkv_dr[:, :N].rearrange("f (y x) -> f y x", x=WP),
                qkv[:3].rearrange("f (y x) -> f y x", x=W)[:, :, :WP])
            qdrf = qkv_dr[:].rearrange("f e -> (f e)")

            # ---------------- grid phase ----------------
            if PHASE < 2:
                xbt0 = small.tile([C, BAND * W], FP, tag="xbt0")
                nc.sync.dma_start(xbt0[:], xband_d[:])
                nc.sync.dma_start(out_d[:], xbt0[:])
            if PHASE < 2:
                raise _PhaseDone()
            gcolt = small.tile([128, 2], FP, tag="gcolt")
            nc.sync.dma_start(gcolt[:], gcol_d[:])
            kb = big.tile([128, N], FP, tag="kb")
            for jq in range(4):
                eng = [nc.sync, nc.gpsimd, nc.sync, nc.gpsimd][jq]
                eng.dma_start(
                    kb[:, jq * JH:(jq + 1) * JH],
                    bass.AP(qdrf.tensor, qdrf.offset + NPAD + jq * JH,
                            [[0, 128], [1, JH]]))
            gvt = small.tile([1, G], FP, tag="gvt")
            nc.sync.dma_start(gvt[:], gvec_d[:])
            ktil = small.tile([128, NPAD // 128], FP, tag="ktil")
            nc.sync.dma_start(
                ktil[:], bass.AP(qdrf.tensor, qdrf.offset + NPAD,
                                 [[1, 128], [128, NPAD // 128]]))
            kmaxp = small.tile([128, 1], FP, tag="kmaxp")
            kminp = small.tile([128, 1], FP, tag="kminp")
            nc.vector.tensor_reduce(kmaxp[:], ktil[:], AX.X, ALU.max)
            nc.vector.tensor_reduce(kminp[:], ktil[:], AX.X, ALU.min)
            khi = small.tile([1, 1], FP, tag="khi")
            nklo = small.tile([1, 1], FP, tag="nklo")
            nc.vector.tensor_scalar_mul(kminp[:], kminp[:], -1.0)
            nc.gpsimd.tensor_reduce(khi[:], kmaxp[:], AX.C, ALU.max)
            nc.gpsimd.tensor_reduce(nklo[:], kminp[:], AX.C, ALU.max)

            bvt = small.tile([128, 1], FP, tag="bvt")
            nc.sync.dma_start(bvt[:], bv_d[:])

            mneg = [small.tile([128, 1], FP, tag=f"mneg{g}", name=f"mneg{g}")
                    for g in range(2)]
            with tc.tile_pool(name="psm", bufs=2, space="PSUM") as psm:
                for gh in range(2):
                    pm = psm.tile([128, 2], FP, tag="pm")
                    gsl = gvt[0:1, gh * 128:(gh + 1) * 128]
                    nc.tensor.matmul(pm[:, 0:1], gsl, khi[:],
                                     start=True, stop=True)
                    nc.tensor.matmul(pm[:, 1:2], gsl, nklo[:],
                                     start=True, stop=True)
                    # mneg = -max(g*khi, g*klo) = min(-g*khi, g*(-klo))
                    t0 = small.tile([128, 1], FP, tag=f"t0_{gh}",
                                    name=f"t0_{gh}")
                    nc.vector.tensor_scalar_mul(t0[:], pm[:, 0:1], -1.0)
                    nc.vector.tensor_tensor(mneg[gh][:], t0[:], pm[:, 1:2],
                                            ALU.min)

            # s1 is dead after stage-2: carve exp/v scratch from it
            etp0 = small.tile([128, JH], FP, tag="etp0")
            etp1 = small.tile([128, JH], FP, tag="etp1")
            vbp0 = small.tile([128, JH], FP, tag="vbp0")
            vbp1 = small.tile([128, JH], FP, tag="vbp1")
            ets = [etp0[:], etp1[:]]
            vbs = [vbp0[:], vbp1[:]]
            wts = [qkv[:, 0:JH], qkv[:, 2048: 2048 + JH]]
            sacc = small.tile([128, 8], FP, tag="sacc")
            wacc = small.tile([128, 8], FP, tag="wacc")
            ctxcol = small.tile([128, 2], FP, tag="ctxcol")
            if True:
                for jq in range(4):
                    j0 = jq * JH
                    vb = vbs[jq % 2]
                    h2 = JH // 2
                    nc.sync.dma_start(
                        vb[:, :h2],
                        bass.AP(qdrf.tensor, qdrf.offset + 2 * NPAD + j0,
                                [[0, 128], [1, h2]]))
                    nc.gpsimd.dma_start(
                        vb[:, h2:],
                        bass.AP(qdrf.tensor,
                                qdrf.offset + 2 * NPAD + j0 + h2,
                                [[0, 128], [1, JH - h2]]))
                    for gh in range(2):
                        et = ets[gh]
                        col = gh * 4 + jq
                        nc.scalar.activation(
                            et, kb[:, j0: j0 + JH], AF.Exp,
                            bias=mneg[gh][:], scale=gcolt[:, gh: gh + 1],
                            accum_out=sacc[:, col: col + 1])
                        wt = wts[gh]
                        nc.vector.tensor_tensor(wt, et, vb, ALU.mult)
                        nc.scalar.activation(
                            et if gh == 0 else vb, wt, AF.Copy,
                            accum_out=wacc[:, col: col + 1])
                for gh in range(2):
                    ssum = small.tile([128, 1], FP, tag=f"ssum{gh}",
                                      name=f"ssum{gh}")
                    wsum = small.tile([128, 1], FP, tag=f"wsum{gh}",
                                      name=f"wsum{gh}")
                    nc.vector.tensor_tensor(
                        ssum[:], sacc[:, gh * 4: gh * 4 + 1],
                        sacc[:, gh * 4 + 1: gh * 4 + 2], ALU.add)
                    nc.vector.tensor_tensor(
                        ssum[:], ssum[:], sacc[:, gh * 4 + 2: gh * 4 + 3],
                        ALU.add)
                    nc.vector.tensor_tensor(
                        ssum[:], ssum[:], sacc[:, gh * 4 + 3: gh * 4 + 4],
                        ALU.add)
                    nc.vector.tensor_tensor(
                        wsum[:], wacc[:, gh * 4: gh * 4 + 1],
                        wacc[:, gh * 4 + 1: gh * 4 + 2], ALU.add)
                    nc.vector.tensor_tensor(
                        wsum[:], wsum[:], wacc[:, gh * 4 + 2: gh * 4 + 3],
                        ALU.add)
                    nc.vector.tensor_tensor(
                        wsum[:], wsum[:], wacc[:, gh * 4 + 3: gh * 4 + 4],
                        ALU.add)
                    nc.vector.reciprocal(ssum[:], ssum[:])
                    nc.vector.tensor_tensor(ctxcol[:, gh: gh + 1], wsum[:],
                                            ssum[:], ALU.mult)
            nc.vector.tensor_scalar_add(ctxcol[:], ctxcol[:], bvt[:])

            # table bounce + partition broadcast
            ctx_dr = dpool.tile([1, G + 4], FP, tag="ctx_dr")
            nc.sync.dma_start(ctx_dr[0:1, 0:128], ctxcol[:, 0:1])
            nc.sync.dma_start(ctx_dr[0:1, 128:256], ctxcol[:, 1:2])
            nc.sync.dma_start(ctx_dr[0:1, 256:260], ctxcol[0:4, 1:2])
            # pre-paired table: table2[p, t, :] = (ctx[t], ctx[t+1])
            table2 = small.tile([128, G, 2], FP, tag="table2")
            cdrf = ctx_dr[:].rearrange("o e -> (o e)")
            nc.sync.dma_start(
                table2[:], bass.AP(cdrf.tensor, cdrf.offset,
                                   [[0, 128], [1, G], [1, 2]]))

            if PHASE < 3:
                xbt0 = small.tile([C, BAND * W], FP, tag="xbt0")
                nc.sync.dma_start(xbt0[:], xband_d[:])
                nc.sync.dma_start(out_d[:], xbt0[:])
                raise _PhaseDone()
            # ---------------- interp phase ----------------
            bqt = small.tile([128, 1], FP, tag="bqt")
            nc.sync.dma_start(bqt[:], bq_d[:])

            # natural [128, 66] chain -> gather indices. The gather reads
            # core c's indices in wrapped (s p) order, so gather position
            # (c, i) maps to pixel pi(c,i) = 66*(16c + i%16) + i//16.
            NW = NEQ // 16
            qw_t = small.tile([128, NW], FP, tag="qw")
            qw = qw_t[:, :NW]
            nc.sync.dma_start(
                qw, bass.AP(qdrf.tensor, qdrf.offset, [[NW, 128], [1, NW]]))
            qp_w = small.tile([128, NW], FP, tag="qp_w")
            nc.scalar.activation(qp_w[:], qw, AF.Identity, bias=bqt[:])
            aa_w = small.tile([128, NW], FP, tag="aa_w")
            nc.scalar.activation(aa_w[:], qp_w[:], AF.Abs, scale=1.0 / C0)
            nc.vector.tensor_scalar_add(aa_w[:], aa_w[:], 1.0)
            nc.vector.reciprocal_approx_fast(out=aa_w[:], in_=aa_w[:])
            ss_w = small.tile([128, NW], FP, tag="ss_w")
            nc.vector.tensor_tensor(ss_w[:], qp_w[:], aa_w[:], ALU.mult)
            nc.vector.tensor_scalar(ss_w[:], ss_w[:], float(ALPHA),
                                    float(BETA), ALU.mult, ALU.add)
            idx32 = small.tile([128, NW], mybir.dt.int32, tag="idx32")
            nc.vector.tensor_copy(idx32[:], ss_w[:])
            idx16 = small.tile([128, NW], mybir.dt.int16, tag="idx16")
            nc.vector.tensor_copy(idx16[:], idx32[:])

            # replicated [128, 1056] chain -> lerp weights (qkv tile is dead:
            # carve scratch from it)
            qr = qkv[:, 0:NEQ]
            for cq in range(8):
                eng = [nc.sync, nc.gpsimd][cq % 2]
                eng.dma_start(
                    qkv[16 * cq: 16 * cq + 16, 0:NEQ],
                    bass.AP(qdrf.tensor, qdrf.offset + NEQ * cq,
                            [[0, 16], [1, NEQ]]))
            qp_r = qkv[:, NEQ: 2 * NEQ]
            nc.scalar.activation(qp_r, qr, AF.Identity, bias=bqt[:])
            aa_r = qkv[:, 2 * NEQ: 3 * NEQ]
            nc.scalar.activation(aa_r, qp_r, AF.Abs, scale=1.0 / C0)
            nc.vector.tensor_scalar_add(aa_r, aa_r, 1.0)
            nc.vector.reciprocal_approx_fast(out=aa_r, in_=aa_r)
            ss_r = qkv[:, 3 * NEQ: 4 * NEQ]
            nc.vector.tensor_tensor(ss_r, qp_r, aa_r, ALU.mult)
            nc.vector.tensor_scalar(ss_r, ss_r, float(ALPHA), float(BETA),
                                    ALU.mult, ALU.add)
            i32r = small.tile([128, NEQ], mybir.dt.int32, tag="i32r")
            nc.vector.tensor_copy(i32r[:], ss_r)
            idxf = qkv[:, 4 * NEQ: 5 * NEQ]
            nc.vector.tensor_copy(idxf, i32r[:])
            fr_r = qkv[:, 7 * NEQ: 8 * NEQ]
            nc.vector.tensor_tensor(fr_r, ss_r, idxf, ALU.subtract)

            gout = small.tile([128, NEQ, 2], FP, tag="gout")
            nc.gpsimd.ap_gather(
                gout[:], table2[:], idx16[:],
                channels=128, num_elems=G, d=2, num_idxs=NEQ)
            # 3-dim free views: iteration pair index i = a*16 + b.
            # gout/dd are stored in wrapped order i; fr_r is stored in
            # per-core pixel order f, and the value for gather slot i lives
            # at f = phi(i) = 66*(i%16) + i//16 -> view "(b a) -> a b".
            # cv is written through the same phi-view, which makes its
            # storage order equal to per-core pixel order -> plain extract.
            g0 = gout[:].rearrange("p (a b) d -> p a b d", b=16)[:, :, :, 0:1].squeeze(3)
            g1 = gout[:].rearrange("p (a b) d -> p a b d", b=16)[:, :, :, 1:2].squeeze(3)
            dd = qkv[:, 5 * NEQ: 6 * NEQ].rearrange("p (a b) -> p a b", b=16)
            fr_v = fr_r.rearrange("p (b a) -> p a b", b=16)
            nc.vector.tensor_tensor(dd, g1, g0, ALU.subtract)
            nc.vector.tensor_tensor(dd, fr_v, dd, ALU.mult)
            cv_t = qkv[:, 6 * NEQ: 7 * NEQ]
            cv_v = cv_t.rearrange("p (b a) -> p a b", b=16)
            nc.vector.tensor_tensor(cv_v, dd, g0, ALU.add)
            cv = cv_t

            ctx2_dr = dpool.tile([8, NEQ], FP, tag="ctx2_dr")
            nc.sync.dma_start(
                ctx2_dr[:],
                cv.rearrange("(q r) i -> q r i", r=16)[:, 0:1, :].squeeze(1))

            if PHASE < 4:
                xbt0 = small.tile([C, BAND * W], FP, tag="xbt0")
                nc.sync.dma_start(xbt0[:], xband_d[:])
                nc.sync.dma_start(out_d[:], xbt0[:])
                raise _PhaseDone()
            # ---------------- epilogue ----------------
            c2f = ctx2_dr[:].rearrange("q i -> (q i)")
            ctx90 = small.tile([HP, HP], FP, tag="ctx90")
            nc.sync.dma_start(
                ctx90[:], bass.AP(c2f.tensor, c2f.offset, [[HP, HP], [1, HP]]))
            ahbt = small.tile([HP, BAND], FP, tag="ahbt")
            nc.sync.dma_start(ahbt[:], ahb_d[:])
            awt = small.tile([HP, W], FP, tag="awt")
            nc.sync.dma_start(awt[:], aw_d[:])
            woutt = small.tile([1, C], FP, tag="woutt")
            nc.sync.dma_start(woutt[:], wout_d[:])
            boutt = small.tile([C, 1], FP, tag="boutt")
            nc.sync.dma_start(boutt[:], bout_d[:])
            xbt = qkv[0:C, 2 * BAND * W: 3 * BAND * W]
            nc.sync.dma_start(xbt, xband_d[:])

            with tc.tile_pool(name="pse", bufs=1, space="PSUM") as pse:
                pe1 = pse.tile([HP, BAND], FP, tag="pe1")
                nc.tensor.matmul(pe1[:], ctx90[:], ahbt[:], start=True,
                                 stop=True)
                t1 = small.tile([HP, BAND], FP, tag="t1")
                nc.vector.tensor_copy(t1[:], pe1[:])
                pe2 = pse.tile([BAND, W], FP, tag="pe2")
                nc.tensor.matmul(pe2[:], t1[:], awt[:], start=True, stop=True)
                up = small.tile([BAND, W], FP, tag="up")
                nc.scalar.copy(up[:], pe2[:])
                up_dr = dpool.tile([1, BAND * W], FP, tag="up_dr")
                nc.sync.dma_start(
                    up_dr[:].rearrange("o (y x) -> (o y) x", x=W), up[:])
                uprow = qkv[0:1, 0: BAND * W]
                nc.sync.dma_start(uprow, up_dr[:])

                pe3 = pse.tile([C, BAND * W], FP, tag="pe3")
                nmm = (BAND * W + 511) // 512
                for s in range(nmm):
                    nsz = min(512, BAND * W - s * 512)
                    nc.tensor.matmul(
                        pe3[:, s * 512: s * 512 + nsz], woutt[:],
                        uprow[:, s * 512: s * 512 + nsz],
                        start=True, stop=True)
                o1 = qkv[0:C, BAND * W: 2 * BAND * W]
                nc.scalar.activation(o1, pe3[:], AF.Identity,
                                     bias=boutt[:])
                nc.vector.tensor_tensor(o1, o1, xbt, ALU.add)
                nc.sync.dma_start(out_d[:], o1)

          except _PhaseDone:
            pass
    nc.compile()
    return nc


def make_in_maps(x, wq, bq, wk, bk, wv, bv, w_out, b_out):
    cdt = mybir.dt.np(CONVDT)
    x = np.ascontiguousarray(np.asarray(x, dtype=np.float32))
    Ah = _resize_mat(HP, H)
    Aw = _resize_mat(WP, W)
    gvec = _grid_values().reshape(1, G)
    gcol = np.ascontiguousarray(
        np.stack([_grid_values()[:128], _grid_values()[128:]], axis=1))
    selw = np.zeros((21, 21), np.float32)
    for dx in range(7):
        for f in range(3):
            selw[f * 7 + dx, 3 * dx + f] = 1.0
    w2 = np.zeros((4, 128, 21), np.float32)
    wf = [np.asarray(w, np.float32).reshape(C, 7, 7) for w in (wq, wk, wv)]
    for g4 in range(4):
        for t in range(2):
            dy = 2 * g4 + t
            if dy > 6:
                continue
            for f in range(3):
                for dx in range(7):
                    w2[g4, t * C:(t + 1) * C, f * 7 + dx] = wf[f][:, dy, dx]
    aw = np.ascontiguousarray(Aw.T)                       # [90, 96]
    wout = np.asarray(w_out, np.float32).reshape(1, C)
    bout = np.asarray(b_out, np.float32).reshape(C, 1)
    bq128 = np.full((128, 1), np.float32(np.asarray(bq).reshape(())),
                    np.float32)
    bv128 = np.full((128, 1), np.float32(np.asarray(bv).reshape(())),
                    np.float32)
    del bk  # k-bias cancels in softmax

    maps = []
    for core in range(8):
        b, g = core // 4, core % 4
        xbp = np.zeros((C, XBF), cdt)
        xbp[:, : H * W] = x[b].reshape(C, H * W).astype(cdt)
        ahb90 = np.ascontiguousarray(Ah[BAND * g: BAND * (g + 1), :].T)
        xband = np.ascontiguousarray(
            x[b][:, BAND * g: BAND * (g + 1), :].reshape(C, BAND * W))
        maps.append({
            "xb": xbp, "xband": xband, "w2": w2.astype(cdt), "gvec": gvec, "gcol": gcol,
            "selw": selw.astype(cdt),
            "ahb90": ahb90, "aw": aw, "wout": wout, "bout": bout,
            "bq128": bq128, "bv128": bv128,
        })
    return maps


def kernel(x, wq, bq, wk, bk, wv, bv, w_out, b_out, **extra):
    global _LAST_RES
    if "nc" not in _NC_CACHE:
        _NC_CACHE["nc"] = build_nc()
    nc = _NC_CACHE["nc"]
    in_maps = make_in_maps(x, wq, bq, wk, bk, wv, bv, w_out, b_out)
    res = run_bass_kernel_spmd(nc, in_maps, core_ids=list(range(8)))
    _LAST_RES = res
    out = np.zeros((B, C, H, W), np.float32)
    for core in range(8):
        b, g = core // 4, core % 4
        out[b, :, BAND * g: BAND * (g + 1), :] = (
            res.results[core]["out"].reshape(C, BAND, W))
    return out

